# revision 7
# baseline (speedup 1.0000x reference)
import math
import sys

import numpy as np

if "/opt/trn_rl_repo" not in sys.path:
    sys.path.insert(0, "/opt/trn_rl_repo")

P = 128
C = 8            # cores
NG = 1024        # graphs
GPC = NG // C    # graphs per core
D1, D2, D3 = 16, 32, 64
DOUT = 256
CH = 64          # tiles per gather/one-hot chunk


def _host_prep(x, edge_index, batch):
    N = x.shape[0]
    src = np.asarray(edge_index[0], np.int64)
    dst = np.asarray(edge_index[1], np.int64)
    b = np.asarray(batch, np.int64)

    sizes = np.bincount(b, minlength=NG)
    csum = np.zeros(NG + 1, np.int64)
    csum[1:] = np.cumsum(sizes)
    core_start = csum[np.arange(C) * GPC]
    core_end = csum[(np.arange(C) + 1) * GPC]
    Nc = core_end - core_start
    NMAXC = int(math.ceil(Nc.max() / P) * P)
    NBLK = NMAXC // P
    NSLOT = C * NBLK

    node_core = np.minimum(b // GPC, C - 1)
    node_local = np.arange(N, dtype=np.int64) - core_start[node_core]

    deg = np.bincount(dst, minlength=N).astype(np.float64) + 1.0
    dinv = (1.0 / np.sqrt(deg)).astype(np.float32)

    ecore = node_core[src]
    slot_g = node_core[dst] * NBLK + node_local[dst] // P
    dcol = node_local[dst] % P

    cnt = np.bincount(ecore * NSLOT + slot_g, minlength=C * NSLOT).reshape(C, NSLOT)
    Tslot = np.maximum(1, np.ceil(cnt.max(axis=0) / P).astype(np.int64))
    tile_base = np.zeros(NSLOT + 1, np.int64)
    tile_base[1:] = np.cumsum(Tslot)
    NT = int(tile_base[-1])
    tile_slot = np.repeat(np.arange(NSLOT), Tslot)
    tile_start = np.zeros(NT, bool)
    tile_start[tile_base[:-1]] = True
    tile_stop = np.zeros(NT, bool)
    tile_stop[tile_base[1:] - 1] = True

    idxA = np.zeros((C, P, NT), np.int32)
    dstfA = np.full((C, P, NT), -1.0, np.float32)
    for c in range(C):
        m = ecore == c
        es, sl, co = src[m], slot_g[m], dcol[m]
        order = np.argsort(sl, kind="stable")
        es, sl, co = es[order], sl[order], co[order]
        srow = es - core_start[c]
        first = np.searchsorted(sl, np.arange(NSLOT), side="left")
        j = np.arange(len(es)) - first[sl]
        t = tile_base[sl] + j // P
        lane = j % P
        idxA[c, lane, t] = srow
        dstfA[c, lane, t] = co

    dinvP = np.zeros((C, P, NBLK), np.float32)
    xTs = np.zeros((C, P, NMAXC), np.float32)
    for c in range(C):
        loc = np.arange(Nc[c])
        dinvP[c, loc % P, loc // P] = dinv[core_start[c]:core_end[c]]
        xTs[c, :, :Nc[c]] = np.asarray(x[core_start[c]:core_end[c]], np.float32).T

    Lmax = int(sizes.max())
    pidx = np.full((C, P, Lmax), NMAXC, np.int32)
    for c in range(C):
        for gi in range(GPC):
            g = c * GPC + gi
            s0 = csum[g] - core_start[c]
            s1 = csum[g + 1] - core_start[c]
            pidx[c, gi, :s1 - s0] = np.arange(s0, s1, dtype=np.int32)

    meta = dict(
        NMAXC=NMAXC, NBLK=NBLK, NT=NT, Lmax=Lmax,
        tile_slot=tile_slot, tile_start=tile_start, tile_stop=tile_stop,
    )
    per_core = dict(idxA=idxA, dstfA=dstfA, dinvP=dinvP, xT=xTs, pidx=pidx)
    return meta, per_core


def _build(nc, meta):
    from concourse import bass, mybir
    import concourse.tile as tile

    FP32 = mybir.dt.float32
    BF16 = mybir.dt.bfloat16
    I32 = mybir.dt.int32
    Alu = mybir.AluOpType
    Act = mybir.ActivationFunctionType

    NMAXC, NBLK, NT, Lmax = meta["NMAXC"], meta["NBLK"], meta["NT"], meta["Lmax"]
    tile_slot = meta["tile_slot"]
    tile_start = meta["tile_start"]
    tile_stop = meta["tile_stop"]

    xT = nc.dram_tensor("xT", [P, NMAXC], FP32, kind="ExternalInput")
    w1 = nc.dram_tensor("w1", [P, D1], FP32, kind="ExternalInput")
    w2 = nc.dram_tensor("w2", [D1, D2], FP32, kind="ExternalInput")
    w3 = nc.dram_tensor("w3", [D2, D3], FP32, kind="ExternalInput")
    l1w = nc.dram_tensor("l1w", [D3, DOUT], FP32, kind="ExternalInput")
    l2w = nc.dram_tensor("l2w", [DOUT, DOUT], FP32, kind="ExternalInput")
    b1t = nc.dram_tensor("b1t", [P, D1], FP32, kind="ExternalInput")
    b2t = nc.dram_tensor("b2t", [P, D2], FP32, kind="ExternalInput")
    b3t = nc.dram_tensor("b3t", [P, D3], FP32, kind="ExternalInput")
    l1bT = nc.dram_tensor("l1bT", [DOUT, 1], FP32, kind="ExternalInput")
    l2bT = nc.dram_tensor("l2bT", [DOUT, 1], FP32, kind="ExternalInput")
    dinvP = nc.dram_tensor("dinvP", [P, NBLK], FP32, kind="ExternalInput")
    idxA = nc.dram_tensor("idxA", [P, NT], I32, kind="ExternalInput")
    dstfA = nc.dram_tensor("dstfA", [P, NT], FP32, kind="ExternalInput")
    pidx = nc.dram_tensor("pidx", [P, Lmax], I32, kind="ExternalInput")
    iota = nc.dram_tensor("iota", [P, P], FP32, kind="ExternalInput")
    out = nc.dram_tensor("out", [GPC, DOUT], FP32, kind="ExternalOutput")

    tab1 = nc.dram_tensor("tab1", [NMAXC, D1], BF16, kind="Internal")
    tab2 = nc.dram_tensor("tab2", [NMAXC, D2], BF16, kind="Internal")
    tab3 = nc.dram_tensor("tab3", [NMAXC, D3], BF16, kind="Internal")
    z3d = nc.dram_tensor("z3d", [NMAXC + P, D3], BF16, kind="Internal")
    part1 = nc.dram_tensor("part1", [C * NMAXC, D1], BF16, kind="Internal")
    part2 = nc.dram_tensor("part2", [C * NMAXC, D2], BF16, kind="Internal")
    part3 = nc.dram_tensor("part3", [C * NMAXC, D3], BF16, kind="Internal")
    rs1 = nc.dram_tensor("rs1", [NMAXC, D1], BF16, kind="Internal")
    rs2 = nc.dram_tensor("rs2", [NMAXC, D2], BF16, kind="Internal")
    rs3 = nc.dram_tensor("rs3", [NMAXC, D3], BF16, kind="Internal")

    from concourse.masks import make_identity

    with tile.TileContext(nc, num_cores=C) as tc:
        with tc.tile_pool(name="res", bufs=1) as rp, \
             tc.tile_pool(name="setup", bufs=1) as sp0:
            # resident constants
            idx_sb = rp.tile([P, NT], I32, name="idx_sb")
            nc.sync.dma_start(idx_sb[:], idxA[:])
            pidx_sb = rp.tile([P, Lmax], I32, name="pidx_sb")
            nc.sync.dma_start(pidx_sb[:], pidx[:])
            dinv_sb = rp.tile([P, NBLK], FP32, name="dinv_sb")
            nc.sync.dma_start(dinv_sb[:], dinvP[:])

            def load_bf16(name, handle, shape):
                tmp = sp0.tile(list(shape), FP32, name=f"{name}_f")
                nc.sync.dma_start(tmp[:], handle[:])
                t = rp.tile(list(shape), BF16, name=name)
                nc.vector.tensor_copy(t[:], tmp[:])
                return t

            dstf_sb = rp.tile([P, NT], FP32, name="dstf_sb")
            nc.sync.dma_start(dstf_sb[:], dstfA[:])
            iota_sb = load_bf16("iota_sb", iota, (P, P))
            w1_sb = load_bf16("w1_sb", w1, (P, D1))
            w2_sb = load_bf16("w2_sb", w2, (D1, D2))
            w3_sb = load_bf16("w3_sb", w3, (D2, D3))
            l1w_sb = load_bf16("l1w_sb", l1w, (D3, DOUT))
            b1t_sb = load_bf16("b1t_sb", b1t, (P, D1))
            b2t_sb = load_bf16("b2t_sb", b2t, (P, D2))
            b3t_sb = load_bf16("b3t_sb", b3t, (P, D3))
            l2w_h = []
            for h in range(2):
                tmp = sp0.tile([P, DOUT], FP32, name=f"l2wf{h}")
                nc.sync.dma_start(tmp[:], l2w[h * P:(h + 1) * P, :])
                t = rp.tile([P, DOUT], BF16, name=f"l2w_sb{h}")
                nc.vector.tensor_copy(t[:], tmp[:])
                l2w_h.append(t)
            l1b_h, l2b_h = [], []
            for h in range(2):
                t = rp.tile([P, 1], FP32, name=f"l1b_sb{h}")
                nc.sync.dma_start(t[:], l1bT[h * P:(h + 1) * P, :])
                l1b_h.append(t)
                t2 = rp.tile([P, 1], FP32, name=f"l2b_sb{h}")
                nc.sync.dma_start(t2[:], l2bT[h * P:(h + 1) * P, :])
                l2b_h.append(t2)

            ident_b = rp.tile([P, P], BF16, name="ident_b")
            make_identity(nc, ident_b[:])
            ident_f = rp.tile([P, P], FP32, name="ident_f")
            make_identity(nc, ident_f[:])

            zero64 = rp.tile([P, D3], BF16, name="zero64")
            nc.vector.memset(zero64[:], 0)
            nc.sync.dma_start(z3d[NMAXC:NMAXC + P, :], zero64[:])

            TT2 = {
                1: rp.tile([P, NBLK * D1], BF16, name="TT2b1"),
                2: rp.tile([P, NBLK * D2], BF16, name="TT2b2"),
                3: rp.tile([P, NBLK * D3], BF16, name="TT2b3"),
            }

            # ---- table 1: T1 = dinv * (x @ W1); TT2b1 = dinv*T1 + b1
            with tc.tile_pool(name="t1", bufs=2) as tp, \
                 tc.tile_pool(name="t1ps", bufs=2, space="PSUM") as pp:
                for bi in range(NBLK):
                    xc = tp.tile([P, P], FP32, tag="xc")
                    nc.sync.dma_start(xc[:], xT[:, bi * P:(bi + 1) * P])
                    xb = tp.tile([P, P], BF16, tag="xb")
                    nc.vector.tensor_copy(xb[:], xc[:])
                    hp = pp.tile([P, 512], FP32, tag="h")
                    nc.tensor.matmul(hp[:, :D1], lhsT=xb[:], rhs=w1_sb[:],
                                     start=True, stop=True)
                    ts = tp.tile([P, D1], BF16, tag="ts")
                    nc.vector.tensor_scalar(out=ts[:], in0=hp[:, :D1],
                                            scalar1=dinv_sb[:, bi:bi + 1],
                                            scalar2=None, op0=Alu.mult)
                    nc.sync.dma_start(tab1[bi * P:(bi + 1) * P, :], ts[:])
                    nc.vector.scalar_tensor_tensor(
                        out=TT2[1][:, bi * D1:(bi + 1) * D1], in0=ts[:],
                        scalar=dinv_sb[:, bi:bi + 1], in1=b1t_sb[:],
                        op0=Alu.mult, op1=Alu.add)

            def conv(tab, part, rs, d):
                nchunks = (NT + CH - 1) // CH
                with tc.tile_pool(name="cm", bufs=2) as mp, \
                     tc.tile_pool(name="cs", bufs=2) as spp, \
                     tc.tile_pool(name="cst", bufs=4) as stp, \
                     tc.tile_pool(name="cacc", bufs=6, space="PSUM") as ap:
                    acc = None
                    for ci in range(nchunks):
                        t0 = ci * CH
                        t1 = min(NT, t0 + CH)
                        W = t1 - t0
                        msg = mp.tile([P, CH * d], BF16, tag="msg")
                        nc.gpsimd.indirect_dma_start(
                            out=msg[:, :W * d], out_offset=None, in_=tab[:],
                            in_offset=bass.IndirectOffsetOnAxis(
                                ap=idx_sb[:, t0:t1], axis=0))
                        stops = [t0 + g for g in range(W) if tile_stop[t0 + g]]
                        s_first = int(tile_slot[stops[0]])
                        s_last = int(tile_slot[stops[-1]])
                        ns = s_last - s_first + 1
                        stage = stp.tile([P, CH * d], BF16, tag="stage")
                        for g in range(W):
                            t = t0 + g
                            S = spp.tile([P, P], BF16, tag="S")
                            nc.vector.tensor_scalar(
                                out=S[:], in0=iota_sb[:],
                                scalar1=dstf_sb[:, t:t + 1], scalar2=None,
                                op0=Alu.is_equal)
                            if tile_start[t]:
                                acc = ap.tile([P, 512], FP32, tag="acc")
                            nc.tensor.matmul(acc[:, :d],
                                             lhsT=S[:],
                                             rhs=msg[:, g * d:(g + 1) * d],
                                             start=bool(tile_start[t]),
                                             stop=bool(tile_stop[t]))
                            if tile_stop[t]:
                                s = int(tile_slot[t])
                                nc.scalar.copy(
                                    stage[:, (s - s_first) * d:(s - s_first + 1) * d],
                                    acc[:, :d])
                        nc.sync.dma_start(
                            out=part[s_first * P:(s_last + 1) * P, :]
                                .rearrange("(s p) d -> p s d", p=P),
                            in_=stage[:, :ns * d]
                                .rearrange("p (s d) -> p s d", d=d))
                nc.gpsimd.collective_compute(
                    "ReduceScatter", Alu.add,
                    replica_groups=[list(range(C))],
                    ins=[part[:].opt()], outs=[rs[:].opt()])

            def epilogue(k, rs, d, nxt):
                # z = relu(dinv*(rs) + TT2[k]); if nxt: build next table, else z3d
                with tc.tile_pool(name=f"ep{k}", bufs=3) as ep, \
                     tc.tile_pool(name=f"eps{k}", bufs=2, space="PSUM") as pp:
                    for bi in range(NBLK):
                        rsb = ep.tile([P, d], BF16, tag="rs")
                        nc.sync.dma_start(rsb[:], rs[bi * P:(bi + 1) * P, :])
                        e = ep.tile([P, d], BF16, tag="e")
                        nc.vector.scalar_tensor_tensor(
                            out=e[:], in0=rsb[:],
                            scalar=dinv_sb[:, bi:bi + 1],
                            in1=TT2[k][:, bi * d:(bi + 1) * d],
                            op0=Alu.mult, op1=Alu.add)
                        z = ep.tile([P, d], BF16, tag="z")
                        nc.scalar.activation(z[:], e[:], Act.Relu)
                        if nxt is None:
                            nc.sync.dma_start(z3d[bi * P:(bi + 1) * P, :], z[:])
                        else:
                            w_n, d_n, tab_n, bt_n = nxt
                            ztp = pp.tile([P, 1024], BF16, tag="zt")
                            nc.tensor.transpose(ztp[:d, :P], z[:], ident_b[:])
                            zts = ep.tile([d, P], BF16, tag="zts")
                            nc.scalar.copy(zts[:], ztp[:d, :P])
                            hp = pp.tile([P, 512], FP32, tag="h")
                            nc.tensor.matmul(hp[:, :d_n], lhsT=zts[:], rhs=w_n[:],
                                             start=True, stop=True)
                            ts = ep.tile([P, d_n], BF16, tag="t")
                            nc.vector.tensor_scalar(
                                out=ts[:], in0=hp[:, :d_n],
                                scalar1=dinv_sb[:, bi:bi + 1],
                                scalar2=None, op0=Alu.mult)
                            nc.sync.dma_start(tab_n[bi * P:(bi + 1) * P, :], ts[:])
                            nc.vector.scalar_tensor_tensor(
                                out=TT2[k + 1][:, bi * d_n:(bi + 1) * d_n],
                                in0=ts[:], scalar=dinv_sb[:, bi:bi + 1],
                                in1=bt_n[:], op0=Alu.mult, op1=Alu.add)

            conv(tab1, part1, rs1, D1)
            epilogue(1, rs1, D1, (w2_sb, D2, tab2, b2t_sb))
            conv(tab2, part2, rs2, D2)
            epilogue(2, rs2, D2, (w3_sb, D3, tab3, b3t_sb))
            conv(tab3, part3, rs3, D3)
            epilogue(3, rs3, D3, None)

            # ---- pooling + MLP
            with tc.tile_pool(name="fin", bufs=1) as fp, \
                 tc.tile_pool(name="fps", bufs=1, space="PSUM") as fpp:
                pg = fp.tile([P, Lmax * D3], BF16, name="pg")
                nc.gpsimd.indirect_dma_start(
                    out=pg[:], out_offset=None, in_=z3d[:],
                    in_offset=bass.IndirectOffsetOnAxis(ap=pidx_sb[:], axis=0))
                pooled = fp.tile([P, D3], FP32, name="pooled")
                pgv = pg[:].rearrange("p (l d) -> p l d", d=D3).transpose([0, 2, 1])
                nc.vector.tensor_reduce(out=pooled[:], in_=pgv,
                                        axis=mybir.AxisListType.X, op=Alu.max)
                poolb = fp.tile([P, D3], BF16, name="poolb")
                nc.vector.tensor_copy(poolb[:], pooled[:])
                ptp = fpp.tile([P, 1024], BF16, tag="ptp")
                nc.tensor.transpose(ptp[:D3, :P], poolb[:], ident_b[:])
                pts = fp.tile([D3, P], BF16, name="pts")
                nc.scalar.copy(pts[:], ptp[:D3, :P])
                p1 = []
                for h in range(2):
                    php = fpp.tile([P, 512], FP32, tag=f"p1{h}")
                    nc.tensor.matmul(php[:, :P], lhsT=l1w_sb[:, h * P:(h + 1) * P],
                                     rhs=pts[:], start=True, stop=True)
                    p1h = fp.tile([P, P], BF16, name=f"p1h{h}")
                    nc.scalar.activation(p1h[:], php[:, :P], Act.Relu,
                                         bias=l1b_h[h][:])
                    p1.append(p1h)
                ots = fp.tile([P, DOUT], FP32, name="ots")
                for h in range(2):
                    p2p = fpp.tile([P, 512], FP32, tag=f"p2{h}")
                    for kb in range(2):
                        nc.tensor.matmul(p2p[:, :P],
                                         lhsT=l2w_h[kb][:, h * P:(h + 1) * P],
                                         rhs=p1[kb][:],
                                         start=(kb == 0), stop=(kb == 1))
                    p2s = fp.tile([P, P], FP32, name=f"p2s{h}")
                    nc.scalar.activation(p2s[:], p2p[:, :P], Act.Sigmoid,
                                         bias=l2b_h[h][:])
                    otp = fpp.tile([P, 512], FP32, tag=f"ot{h}")
                    nc.tensor.transpose(otp[:, :P], p2s[:], ident_f[:])
                    nc.scalar.copy(ots[:, h * P:(h + 1) * P], otp[:, :P])
                nc.sync.dma_start(out[:], ots[:])


def _in_maps(inputs, pc):
    iota_np = np.broadcast_to(np.arange(P, dtype=np.float32), (P, P)).copy()
    shared = {
        "w1": np.asarray(inputs["W1"], np.float32),
        "w2": np.asarray(inputs["W2"], np.float32),
        "w3": np.asarray(inputs["W3"], np.float32),
        "l1w": np.asarray(inputs["L1_w"], np.float32),
        "l2w": np.asarray(inputs["L2_w"], np.float32),
        "b1t": np.broadcast_to(np.asarray(inputs["b1"], np.float32), (P, D1)).copy(),
        "b2t": np.broadcast_to(np.asarray(inputs["b2"], np.float32), (P, D2)).copy(),
        "b3t": np.broadcast_to(np.asarray(inputs["b3"], np.float32), (P, D3)).copy(),
        "l1bT": np.asarray(inputs["L1_b"], np.float32).reshape(DOUT, 1),
        "l2bT": np.asarray(inputs["L2_b"], np.float32).reshape(DOUT, 1),
        "iota": iota_np,
    }
    in_maps = []
    for c in range(C):
        m = dict(shared)
        m["xT"] = np.ascontiguousarray(pc["xT"][c])
        m["idxA"] = np.ascontiguousarray(pc["idxA"][c])
        m["dstfA"] = np.ascontiguousarray(pc["dstfA"][c])
        m["dinvP"] = np.ascontiguousarray(pc["dinvP"][c])
        m["pidx"] = np.ascontiguousarray(pc["pidx"][c])
        in_maps.append(m)
    return in_maps


def build_and_run(inputs, trace=False, trace_kwargs=None):
    from concourse import bacc, bass_utils

    x = np.asarray(inputs["x"], np.float32)
    meta, pc = _host_prep(x, np.asarray(inputs["edge_index"]),
                          np.asarray(inputs["batch"]))

    nc = bacc.Bacc("TRN2", target_bir_lowering=False, debug=False, num_devices=C)
    _build(nc, meta)
    nc.compile()

    in_maps = _in_maps(inputs, pc)
    res = bass_utils.run_bass_kernel_spmd(
        nc, in_maps, core_ids=list(range(C)),
        trace=trace, **(trace_kwargs or {}))
    full = np.concatenate(
        [np.asarray(r["out"], np.float32) for r in res.results], axis=0)
    return full, res


def kernel(**inputs):
    out, _ = build_and_run(inputs, trace=False)
    return out
